# revision 14
# baseline (speedup 1.0000x reference)
"""Multi-head attention (B=4, S=2048, D=1024, H=16, d_k=64) on 8 TRN2 NeuronCores.

Sharding: batch x head-group. Core c handles batch b = c//2 and heads
[8*(c%2), 8*(c%2)+8). Each core computes Q/K/V projections for its 512
output features (column-parallel), attention for its 8 heads, and a
row-parallel partial of the W_o output projection. The host sums the two
bf16 partials per batch (the row-parallel unshard) — no collectives.

The kernel is ACT-bound: softmax needs 8*2048^2 = 33.5M exps per core at
1 elem/cycle/lane @1.2GHz, ~1.08us per 128x1024 kb-tile, ~277us total.
Every engine runs its queue IN ORDER, so the schedule keeps the ACT exp
stream dense:

- Slot t emits exp(t), then scores(t+1), then attn@V(t-1). Deferring AV
  by one slot means the PE never waits on ACT (pt(t-1) is ready), and
  scores(t+1) is always produced a full slot before exp(t+1) needs it.
- All other PE work (projection chunks, W_o tiles, norm matmuls) flows
  through a deadline-sorted queue released in ~0.45us micro-steps into
  each slot's PE slack, force-released when a deadline is due.
- Inputs arrive as ~16 large multi-block DMAs (one dma_start spreads
  over all 16 SDMA engines); non-critical ones are WAR-gated behind the
  2.5MB that scores(0) needs, so the ramp is ~8us not ~25us.
- m-outer / qc-inner; W_o for chunk qc runs inside the m3 phase one qc
  behind its normalization; output partials stream out as bf16 halves.
- Softmax denominators (65th ones-column of V_aug) are pulled straight
  from PSUM row 64 via DMA and inverted with reciprocal_approx_fast
  (~5x faster than the stock reciprocal; den ~ 1e3, well inside range).

Inner loop per kb: scores^T via two concurrent row-tiled K=64 matmuls
(tile_position (0,0)/(64,0)), one ACT exp (scale=1/8, max-subtraction
skipped since scores ~ N(0,1)), attn@V as two M=65 matmuls.
"""

import os
from functools import partial

import numpy as np
import ml_dtypes

import concourse.bacc as bacc
import concourse.mybir as mybir
import concourse.tile as tile
from concourse.bass_utils import run_bass_kernel_spmd

BF16 = mybir.dt.bfloat16
F32 = mybir.dt.float32
EXP = mybir.ActivationFunctionType.Exp

B, S, D = 4, 2048, 1024
H, DK = 16, 64
HPC = 8           # heads per core
FPC = HPC * DK    # 512 features per core
NP = 4            # head pairs per core
NB = 8            # din blocks of 128
NKB = 16          # key blocks of 128
NQC = 4           # q chunks of 512
QC = 512
NTT = 16          # token tiles of 128

_nc_cache = None
last_results = None


def gslot(m, qc, kb):
    return (m * NQC + qc) * NKB + kb


def build():
    nc = bacc.Bacc("TRN2", target_bir_lowering=False, debug=False, num_devices=8)

    xq = nc.dram_tensor("xq", [D, S], BF16, kind="ExternalInput").ap()
    xk = nc.dram_tensor("xk", [D, S], BF16, kind="ExternalInput").ap()
    xv = nc.dram_tensor("xv", [D, S], BF16, kind="ExternalInput").ap()
    wq = nc.dram_tensor("wq", [D, FPC], BF16, kind="ExternalInput").ap()
    wk = nc.dram_tensor("wk", [D, FPC], BF16, kind="ExternalInput").ap()
    wv = nc.dram_tensor("wv", [D, FPC], BF16, kind="ExternalInput").ap()
    wo = nc.dram_tensor("wo", [FPC, D], BF16, kind="ExternalInput").ap()
    mask = nc.dram_tensor("mask", [2, 128], BF16, kind="ExternalInput").ap()
    out = nc.dram_tensor("out", [S, D], BF16, kind="ExternalOutput").ap()

    with tile.TileContext(nc) as tc:
        with (
            tc.tile_pool(name="wp", bufs=1) as wp,
            tc.tile_pool(name="qkv", bufs=1) as qkv,
            tc.tile_pool(name="xp", bufs=3) as xp,
            tc.tile_pool(name="ptp", bufs=2) as ptp,
            tc.tile_pool(name="otp", bufs=4) as otp,
            tc.tile_pool(name="smalls", bufs=2) as smalls,
            tc.tile_pool(name="sm1", bufs=1) as sm1,
            tc.tile_pool(name="outp", bufs=2) as outp,
            tc.tile_pool(name="sp", bufs=2, space="PSUM") as sp,
            tc.tile_pool(name="avp", bufs=2, space="PSUM") as avp,
            tc.tile_pool(name="miscp", bufs=2, space="PSUM") as miscp,
        ):
            wq_sb = wp.tile([128, NB, NP, 128], BF16, tag="wq")
            wk_sb = wp.tile([128, NB, NP, 128], BF16, tag="wk")
            wv_sb = wp.tile([128, NB, FPC], BF16, tag="wv")
            wo_sb = wp.tile([128, NP, D], BF16, tag="wo")
            m_sb = wp.tile([2, 128], BF16, tag="mask")

            qt_sb = qkv.tile([128, NP, S], BF16, tag="qt")
            kt_sb = qkv.tile([128, NP, S], BF16, tag="kt")
            v_sb = qkv.tile([128, NKB, HPC, 65], BF16, tag="v")

            xq_sb = xp.tile([128, NB, S], BF16, tag="x", name="xq_sb")
            xk_sb = xp.tile([128, NB, S], BF16, tag="x", name="xk_sb")
            xv_sb = xp.tile([128, NB, S], BF16, tag="x", name="xv_sb")

            nc.sync.dma_start(m_sb[:], mask)
            nc.vector.memset(v_sb[:, :, :, 64], 1.0)

            # ---- consolidated DMAs, ordered by first use ----
            # Later DMAs are WAR-gated (tiny DVE corner write creating a
            # write-after-write ordering) so the 2.5MB scores(0) needs gets
            # the SDMA engines to itself first.
            def xsrc(x_, c0, c1):
                return x_[:, c0:c1].rearrange("(b p) c -> p b c", p=128)

            def gated(dst_corner, src_corner):
                nc.vector.tensor_copy(dst_corner, src_corner)

            kc = xk_sb[0:1, 0, 0:2]
            qc_ = xq_sb[0:1, 0, 0:2]
            nc.sync.dma_start(xk_sb[:, :, 0:512], xsrc(xk, 0, 512))
            nc.sync.dma_start(wk_sb[:, :, 0],
                              wk[:, 0:128].rearrange("(b p) c -> p b c", p=128))
            nc.sync.dma_start(xq_sb[:, :, 0:512], xsrc(xq, 0, 512))
            nc.sync.dma_start(wq_sb[:, :, 0],
                              wq[:, 0:128].rearrange("(b p) c -> p b c", p=128))
            gated(wv_sb[0:1, 0, 0:2], kc)
            nc.sync.dma_start(wv_sb[:, :, 0:128],
                              wv[:, 0:128].rearrange("(b p) c -> p b c", p=128))
            gated(xv_sb[0:1, 0, 0:2], kc)
            nc.sync.dma_start(xv_sb[:, :, 0:256], xsrc(xv, 0, 256))
            gated(xk_sb[0:1, 0, 512:514], qc_)
            nc.sync.dma_start(xk_sb[:, :, 512:1024], xsrc(xk, 512, 1024))
            gated(xv_sb[0:1, 0, 256:258], qc_)
            nc.sync.dma_start(xv_sb[:, :, 256:768], xsrc(xv, 256, 768))
            gated(xk_sb[0:1, 0, 1024:1026], qc_)
            nc.sync.dma_start(xk_sb[:, :, 1024:2048], xsrc(xk, 1024, 2048))
            gated(xv_sb[0:1, 0, 768:770], qc_)
            nc.sync.dma_start(xv_sb[:, :, 768:1536], xsrc(xv, 768, 1536))
            gated(xv_sb[0:1, 0, 1536:1538], qc_)
            nc.sync.dma_start(xv_sb[:, :, 1536:2048], xsrc(xv, 1536, 2048))
            gated(xq_sb[0:1, 0, 512:514], qc_)
            nc.sync.dma_start(xq_sb[:, :, 512:2048], xsrc(xq, 512, 2048))
            gated(wk_sb[0:1, 0, 1, 0:2], qc_)
            nc.sync.dma_start(
                wk_sb[:, :, 1:4],
                wk[:, 128:512].rearrange("(b p) (m c) -> p b m c", p=128, c=128))
            gated(wq_sb[0:1, 0, 1, 0:2], qc_)
            nc.sync.dma_start(
                wq_sb[:, :, 1:4],
                wq[:, 128:512].rearrange("(b p) (m c) -> p b m c", p=128, c=128))
            gated(wv_sb[0:1, 0, 128:130], qc_)
            nc.sync.dma_start(wv_sb[:, :, 128:512],
                              wv[:, 128:512].rearrange("(b p) c -> p b c", p=128))
            gated(wo_sb[0:1, 0, 0:2], qc_)
            nc.sync.dma_start(wo_sb[:],
                              wo.rearrange("(fb p) j -> p fb j", p=128))

            # ---- micro-step emitters (misc PSUM rotates chunk-atomically) ----
            state = {}

            def kq_step(x_sb, w_sb, dst, m, c, i):
                if i == 0:
                    state["kq"] = miscp.tile([128, 512], F32, tag="misc", name="projc")
                ps = state["kq"]
                for b in (2 * i, 2 * i + 1):
                    nc.tensor.matmul(
                        ps[:], w_sb[:, b, m], x_sb[:, b, c * 512:(c + 1) * 512],
                        start=(b == 0), stop=(b == NB - 1))
                if i == 3:
                    nc.vector.tensor_copy(dst[:, m, c * 512:(c + 1) * 512], ps[:])

            def v_step(m, tt, i):
                if i == 0:
                    state[("v", m, tt)] = miscp.tile([128, 512], F32, tag="misc", name="vc")
                ps = state[("v", m, tt)]
                for b in range(4 * i, 4 * i + 4):
                    nc.tensor.matmul(
                        ps[:, 0:128], xv_sb[:, b, tt * 128:(tt + 1) * 128],
                        wv_sb[:, b, m * 128:(m + 1) * 128],
                        start=(b == 0), stop=(b == NB - 1))
                if i == 1:
                    nc.vector.tensor_copy(
                        v_sb[:, tt, 2 * m:2 * m + 2, 0:64],
                        ps[:, 0:128].rearrange("p (h c) -> p h c", c=64))
                    del state[("v", m, tt)]

            ot_tiles = {qc: otp.tile([128, NP, QC], BF16, tag="ot", name=f"ot{qc}")
                        for qc in range(NQC)}

            def finish_pair(job):
                ot_t, m_t, av_sb, rec2 = job
                scp = miscp.tile([128, QC], F32, tag="misc", name="scp")
                nc.tensor.matmul(scp[:], m_sb[:], rec2[:], start=True, stop=True)
                nc.vector.tensor_mul(ot_t[0:64, m_t], av_sb[0:64, 0:QC], scp[0:64, :])
                nc.vector.tensor_mul(ot_t[64:128, m_t], av_sb[0:64, QC:2 * QC], scp[64:128, :])

            def wo_step(qc_w, tt, jc, i):
                if i == 0:
                    state["wo"] = miscp.tile([128, QC], F32, tag="misc", name="wop")
                wop = state["wo"]
                ot_w = ot_tiles[qc_w]
                tsl = slice(tt * 128, (tt + 1) * 128)
                for fb in (2 * i, 2 * i + 1):
                    nc.tensor.matmul(
                        wop[:], ot_w[:, fb, tsl], wo_sb[:, fb, jc * 512:(jc + 1) * 512],
                        start=(fb == 0), stop=(fb == NP - 1))
                if i == 1:
                    ostage = outp.tile([128, QC], BF16, tag="ostage", name="ostage")
                    nc.vector.tensor_copy(ostage[:], wop[:])
                    row = qc_w * QC + tt * 128
                    nc.sync.dma_start(
                        out[row:row + 128, jc * 512:(jc + 1) * 512], ostage[:])

            # ---- deadline-driven work queue (consumed FIFO, built sorted) ----
            work = []

            def push(deadline, cost, fn):
                work.append((deadline, cost, fn))

            def drain(g, budget=470):
                spent = 0
                while work:
                    d, cost, fn = work[0]
                    if d > g + 1 and spent + cost > budget:
                        break
                    work.pop(0)
                    fn()
                    spent += cost

            MARGIN = 2
            items = []
            for m in range(NP):
                for c in range(4):
                    if m == 0 and c == 0:
                        continue
                    items.append((gslot(m, 0, 4 * c) - MARGIN, "k", m, c))
                for c in range(4):
                    if m == 0 and c == 0:
                        continue
                    items.append((gslot(m, c, 0) - MARGIN, "q", m, c))
                for tt in range(NTT):
                    if m == 0 and tt < 2:
                        continue
                    items.append((gslot(m, 0, tt) - 1, "v", m, tt))
            items.sort(key=lambda it: it[0])
            for d, kind, m, x in items:
                if kind == "k":
                    for i in range(4):
                        push(d - (3 - i), 450, partial(kq_step, xk_sb, wk_sb, kt_sb, m, x, i))
                elif kind == "q":
                    for i in range(4):
                        push(d - (3 - i), 450, partial(kq_step, xq_sb, wq_sb, qt_sb, m, x, i))
                else:
                    for i in range(2):
                        push(d - (1 - i), 350, partial(v_step, m, x, i))

            # ---- prefix: K m0 c0, Q m0 c0, then scores(0) ----
            for i in range(4):
                kq_step(xk_sb, wk_sb, kt_sb, 0, 0, i)
            for i in range(4):
                kq_step(xq_sb, wq_sb, qt_sb, 0, 0, i)

            def emit_scores(m, qc, kb):
                s = sp.tile([128, 1024], F32, tag="s", name="s")
                ksl = slice(kb * 128, (kb + 1) * 128)
                qsl = slice(qc * QC, (qc + 1) * QC)
                nc.tensor.matmul(s[:, 0:512], kt_sb[0:64, m, ksl], qt_sb[0:64, m, qsl],
                                 start=True, stop=True, tile_position=(0, 0))
                nc.tensor.matmul(s[:, 512:1024], kt_sb[64:128, m, ksl], qt_sb[64:128, m, qsl],
                                 start=True, stop=True, tile_position=(64, 0))
                return s

            cur = {}

            def emit_av(m, qc, kb, pt):
                if kb == 0:
                    cur["avA"] = avp.tile([128, QC], F32, tag="av", name="avA")
                    cur["avB"] = avp.tile([128, QC], F32, tag="av", name="avB")
                nc.tensor.matmul(cur["avA"][0:65, :], v_sb[:, kb, 2 * m, 0:65],
                                 pt[:, 0:512],
                                 start=(kb == 0), stop=(kb == NKB - 1))
                nc.tensor.matmul(cur["avB"][0:65, :], v_sb[:, kb, 2 * m + 1, 0:65],
                                 pt[:, 512:1024],
                                 start=(kb == 0), stop=(kb == NKB - 1))

            def evac_pair(m, qc):
                avA, avB = cur["avA"], cur["avB"]
                av_sb = smalls.tile([128, 1024], BF16, tag="av_sb", name="av_sb")
                nc.vector.tensor_copy(av_sb[0:65, 0:QC], avA[0:65, :])
                nc.vector.tensor_copy(av_sb[0:65, QC:2 * QC], avB[0:65, :])
                den2 = sm1.tile([2, QC], BF16, tag="den2", name="den2")
                nc.sync.dma_start(den2[0:2, :], av_sb[64:65, 0:2 * QC])
                denf = sm1.tile([2, QC], F32, tag="denf", name="denf")
                nc.vector.tensor_copy(denf[:], den2[:])
                nc.vector.reciprocal_approx_fast(denf[:], denf[:])
                rec2 = smalls.tile([2, QC], BF16, tag="rec2", name="rec2")
                nc.vector.tensor_copy(rec2[:], denf[:])
                return (ot_tiles[qc], m, av_sb, rec2)

            SLOTS = [(m, qc, kb) for m in range(NP) for qc in range(NQC)
                     for kb in range(NKB)]
            s_cur = emit_scores(0, 0, 0)
            pending = None
            prev = None
            for t, (m, qc, kb) in enumerate(SLOTS):
                pt = ptp.tile([128, 1024], BF16, tag="pt", name="pt")
                nc.scalar.activation(pt[:], s_cur[:], EXP, scale=0.125)
                if t + 1 < len(SLOTS):
                    s_cur = emit_scores(*SLOTS[t + 1])
                if t == 0:
                    # V tt0/tt1 land between the first exp and AV(0)
                    for i in range(2):
                        v_step(0, 0, i)
                    for i in range(2):
                        v_step(0, 1, i)
                if prev is not None:
                    pm, pqc, pkb, ppt = prev
                    emit_av(pm, pqc, pkb, ppt)
                    if pkb == NKB - 1:
                        job = evac_pair(pm, pqc)
                        if pm < NP - 1 and pending is not None:
                            finish_pair(pending)
                        pending = job
                if kb == 0 and m == NP - 1:
                    # m3: flush the previous pair's norm mid-pair and pack the
                    # previous q chunk's W_o one step per slot
                    pj = pending
                    pending = None
                    g0 = gslot(m, qc, 0)
                    push(g0 + 4, 250, partial(finish_pair, pj))
                    if qc > 0:
                        for k in range(16):
                            j, jc, i = k // 4, (k // 2) % 2, k % 2
                            push(g0 + 5 + (10 * k) // 16, 450,
                                 partial(wo_step, qc - 1, j, jc, i))
                prev = (m, qc, kb, pt)
                drain(t)

            # drain: last AV, last pair's normalization, last q chunk's W_o
            pm, pqc, pkb, ppt = prev
            emit_av(pm, pqc, pkb, ppt)
            job = evac_pair(pm, pqc)
            drain(10 ** 9, budget=10 ** 9)
            finish_pair(job)
            for tt in range(4):
                for jc in range(2):
                    for i in range(2):
                        wo_step(NQC - 1, tt, jc, i)

    nc.compile()
    return nc


def _get_nc():
    global _nc_cache
    if _nc_cache is None:
        _nc_cache = build()
    return _nc_cache


def kernel(query, key, value, W_q, W_k, W_v, W_o):
    global last_results
    nc = _get_nc()
    bf = ml_dtypes.bfloat16

    mask = np.zeros((2, 128), bf)
    mask[0, 0:64] = 1.0
    mask[1, 64:128] = 1.0

    in_maps = []
    xt = {}
    for b in range(B):
        xt[b] = {
            "xq": np.ascontiguousarray(query[b].T).astype(bf),
            "xk": np.ascontiguousarray(key[b].T).astype(bf),
            "xv": np.ascontiguousarray(value[b].T).astype(bf),
        }
    wmaps = []
    for hg in range(2):
        r = slice(hg * FPC, (hg + 1) * FPC)
        wmaps.append({
            "wq": np.ascontiguousarray(W_q[r, :].T).astype(bf),
            "wk": np.ascontiguousarray(W_k[r, :].T).astype(bf),
            "wv": np.ascontiguousarray(W_v[r, :].T).astype(bf),
            "wo": np.ascontiguousarray(W_o[:, r].T).astype(bf),
        })
    for c in range(8):
        b, hg = c // 2, c % 2
        in_maps.append({**xt[b], **wmaps[hg], "mask": mask})

    res = run_bass_kernel_spmd(
        nc, in_maps, core_ids=list(range(8)),
        trace=bool(os.environ.get("BASS_KERNEL_TRACE")))
    last_results = res

    out = np.empty((B, S, D), np.float32)
    for b in range(B):
        out[b] = (res.results[2 * b]["out"].astype(np.float32)
                  + res.results[2 * b + 1]["out"].astype(np.float32))
    return out


# revision 28
# speedup vs baseline: 1.0140x; 1.0140x over previous
"""Multi-head attention (B=4, S=2048, D=1024, H=16, d_k=64) on 8 TRN2 NeuronCores.

Sharding: batch x head-group. Core c handles batch b = c//2 and heads
[8*(c%2), 8*(c%2)+8). Each core computes Q/K/V projections for its 512
output features (column-parallel), attention for its 8 heads, and a
row-parallel partial of the W_o output projection. The host sums the two
bf16 partials per batch (the row-parallel unshard) — no collectives.

The kernel is ACT-bound: softmax needs 8*2048^2 = 33.5M exps per core at
1 elem/cycle/lane @1.2GHz, ~1.08us per 128x1024 kb-tile, ~277us total.
Every engine runs its queue IN ORDER, so the schedule keeps the ACT exp
stream dense:

- Slot t emits exp(t), then scores(t+1), then attn@V(t-1). Deferring AV
  by one slot means the PE never waits on ACT (pt(t-1) is ready), and
  scores(t+1) is always produced a full slot before exp(t+1) needs it.
- All other PE work (projection chunks, W_o tiles, norm matmuls) flows
  through a deadline-sorted queue released in ~0.45us micro-steps into
  each slot's PE slack, force-released when a deadline is due.
- Inputs arrive as ~16 large multi-block DMAs (one dma_start spreads
  over all 16 SDMA engines); non-critical ones are WAR-gated behind the
  2.5MB that scores(0) needs, so the ramp is ~8us not ~25us.
- m-outer / qc-inner; W_o for chunk qc runs inside the m3 phase one qc
  behind its normalization; output partials stream out as bf16 halves.
- Softmax denominators (65th ones-column of V_aug) are pulled straight
  from PSUM row 64 via DMA and inverted with reciprocal_approx_fast
  (~5x faster than the stock reciprocal; den ~ 1e3, well inside range).

Inner loop per kb: scores^T via two concurrent row-tiled K=64 matmuls
(tile_position (0,0)/(64,0)), one ACT exp (scale=1/8, max-subtraction
skipped since scores ~ N(0,1)), attn@V as two M=65 matmuls.
"""

import os
from functools import partial

import numpy as np
import ml_dtypes

import concourse.bacc as bacc
import concourse.mybir as mybir
import concourse.tile as tile
from concourse.bass_utils import run_bass_kernel_spmd

BF16 = mybir.dt.bfloat16
F32 = mybir.dt.float32
EXP = mybir.ActivationFunctionType.Exp

B, S, D = 4, 2048, 1024
H, DK = 16, 64
HPC = 8           # heads per core
FPC = HPC * DK    # 512 features per core
NP = 4            # head pairs per core
NB = 8            # din blocks of 128
NKB = 16          # key blocks of 128
NQC = 4           # q chunks of 512
QC = 512
NTT = 16          # token tiles of 128

_nc_cache = None
last_results = None


def gslot(m, qc, kb):
    return (m * NQC + qc) * NKB + kb


def build():
    nc = bacc.Bacc("TRN2", target_bir_lowering=False, debug=False, num_devices=8)

    xq = nc.dram_tensor("xq", [D, S], BF16, kind="ExternalInput").ap()
    xk = nc.dram_tensor("xk", [D, S], BF16, kind="ExternalInput").ap()
    xv = nc.dram_tensor("xv", [D, S], BF16, kind="ExternalInput").ap()
    wq = nc.dram_tensor("wq", [D, FPC], BF16, kind="ExternalInput").ap()
    wk = nc.dram_tensor("wk", [D, FPC], BF16, kind="ExternalInput").ap()
    wv = nc.dram_tensor("wv", [D, FPC], BF16, kind="ExternalInput").ap()
    wo = nc.dram_tensor("wo", [FPC, D], BF16, kind="ExternalInput").ap()
    out = nc.dram_tensor("out", [S, D], BF16, kind="ExternalOutput").ap()

    with tile.TileContext(nc) as tc:
        with (
            tc.tile_pool(name="wp", bufs=1) as wp,
            tc.tile_pool(name="qkv", bufs=1) as qkv,
            tc.tile_pool(name="xp", bufs=3) as xp,
            tc.tile_pool(name="ptp", bufs=3) as ptp,
            tc.tile_pool(name="otp", bufs=4) as otp,
            tc.tile_pool(name="smalls", bufs=2) as smalls,
            tc.tile_pool(name="outp", bufs=2) as outp,
            tc.tile_pool(name="sp", bufs=2, space="PSUM") as sp,
            tc.tile_pool(name="avp", bufs=2, space="PSUM") as avp,
            tc.tile_pool(name="miscp", bufs=2, space="PSUM") as miscp,
        ):
            wq_sb = wp.tile([128, NB, NP, 128], BF16, tag="wq")
            wk_sb = wp.tile([128, NB, NP, 128], BF16, tag="wk")
            wv_sb = wp.tile([128, NB, FPC], BF16, tag="wv")
            wo_sb = wp.tile([128, NP, D], BF16, tag="wo")
            ones65 = wp.tile([65, 128], BF16, tag="ones65")
            nc.vector.memset(ones65[:], 1.0)

            qt_sb = qkv.tile([128, NP, S], BF16, tag="qt")
            kt_sb = qkv.tile([128, NP, S], BF16, tag="kt")
            v_sb = qkv.tile([128, NKB, HPC, 65], BF16, tag="v")

            xq_sb = xp.tile([128, NB, S], BF16, tag="x", name="xq_sb")
            xk_sb = xp.tile([128, NB, S], BF16, tag="x", name="xk_sb")
            xv_sb = xp.tile([128, NB, S], BF16, tag="x", name="xv_sb")

            nc.vector.memset(v_sb[:, :, :, 64], 1.0)

            # ---- consolidated DMAs, ordered by first use ----
            # Later DMAs are WAR-gated (tiny DVE corner write creating a
            # write-after-write ordering) so the 2.5MB scores(0) needs gets
            # the SDMA engines to itself first.
            def xsrc(x_, c0, c1):
                return x_[:, c0:c1].rearrange("(b p) c -> p b c", p=128)

            def gated(dst_corner, src_corner):
                nc.vector.tensor_copy(dst_corner, src_corner)

            kc = xk_sb[0:1, 0, 0:2]
            qc_ = xq_sb[0:1, 0, 0:2]
            nc.sync.dma_start(xq_sb[:, :, 0:512], xsrc(xq, 0, 512))
            nc.sync.dma_start(wq_sb[:, :, 0],
                              wq[:, 0:128].rearrange("(b p) c -> p b c", p=128))
            nc.sync.dma_start(xk_sb[:, :, 0:512], xsrc(xk, 0, 512))
            nc.sync.dma_start(wk_sb[:, :, 0],
                              wk[:, 0:128].rearrange("(b p) c -> p b c", p=128))
            gated(wv_sb[0:1, 0, 0:2], kc)
            nc.sync.dma_start(wv_sb[:, :, 0:128],
                              wv[:, 0:128].rearrange("(b p) c -> p b c", p=128))
            gated(xv_sb[0:1, 0, 0:2], kc)
            nc.sync.dma_start(xv_sb[:, :, 0:256], xsrc(xv, 0, 256))
            gated(xk_sb[0:1, 0, 512:514], qc_)
            nc.sync.dma_start(xk_sb[:, :, 512:1024], xsrc(xk, 512, 1024))
            gated(xv_sb[0:1, 0, 256:258], qc_)
            nc.sync.dma_start(xv_sb[:, :, 256:768], xsrc(xv, 256, 768))
            gated(xk_sb[0:1, 0, 1024:1026], qc_)
            nc.sync.dma_start(xk_sb[:, :, 1024:2048], xsrc(xk, 1024, 2048))
            gated(xv_sb[0:1, 0, 768:770], qc_)
            nc.sync.dma_start(xv_sb[:, :, 768:1536], xsrc(xv, 768, 1536))
            gated(xv_sb[0:1, 0, 1536:1538], qc_)
            nc.sync.dma_start(xv_sb[:, :, 1536:2048], xsrc(xv, 1536, 2048))
            gated(xq_sb[0:1, 0, 512:514], qc_)
            nc.sync.dma_start(xq_sb[:, :, 512:2048], xsrc(xq, 512, 2048))
            gated(wk_sb[0:1, 0, 1, 0:2], qc_)
            nc.sync.dma_start(
                wk_sb[:, :, 1:4],
                wk[:, 128:512].rearrange("(b p) (m c) -> p b m c", p=128, c=128))
            gated(wq_sb[0:1, 0, 1, 0:2], qc_)
            nc.sync.dma_start(
                wq_sb[:, :, 1:4],
                wq[:, 128:512].rearrange("(b p) (m c) -> p b m c", p=128, c=128))
            gated(wv_sb[0:1, 0, 128:130], qc_)
            nc.sync.dma_start(wv_sb[:, :, 128:512],
                              wv[:, 128:512].rearrange("(b p) c -> p b c", p=128))
            gated(wo_sb[0:1, 0, 0:2], qc_)
            nc.sync.dma_start(wo_sb[:],
                              wo.rearrange("(fb p) j -> p fb j", p=128))

            # ---- micro-step emitters (misc PSUM rotates chunk-atomically) ----
            state = {}

            def kq_step(x_sb, w_sb, dst, m, c, i):
                if i == 0:
                    state["kq"] = miscp.tile([128, 512], F32, tag="misc", name="projc")
                ps = state["kq"]
                for b in (2 * i, 2 * i + 1):
                    nc.tensor.matmul(
                        ps[:], w_sb[:, b, m], x_sb[:, b, c * 512:(c + 1) * 512],
                        start=(b == 0), stop=(b == NB - 1))
                if i == 3:
                    nc.vector.tensor_copy(dst[:, m, c * 512:(c + 1) * 512], ps[:])

            def v_step(m, tt, i):
                if i == 0:
                    state[("v", m, tt)] = miscp.tile([128, 512], F32, tag="misc", name="vc")
                ps = state[("v", m, tt)]
                for b in range(4 * i, 4 * i + 4):
                    nc.tensor.matmul(
                        ps[:, 0:128], xv_sb[:, b, tt * 128:(tt + 1) * 128],
                        wv_sb[:, b, m * 128:(m + 1) * 128],
                        start=(b == 0), stop=(b == NB - 1))
                if i == 1:
                    nc.vector.tensor_copy(
                        v_sb[:, tt, 2 * m:2 * m + 2, 0:64],
                        ps[:, 0:128].rearrange("p (h c) -> p h c", c=64))
                    del state[("v", m, tt)]

            ot_tiles = {qc: otp.tile([128, NP, QC], BF16, tag="ot", name=f"ot{qc}")
                        for qc in range(NQC)}

            def finish_pair(job):
                # broadcast each head's den row across partitions with a K=1
                # ones matmul, invert in place in PSUM, multiply — no DMA and
                # no SBUF scratch anywhere in the chain
                ot_t, m_t, av_sb = job
                for h in (0, 1):
                    dbc = miscp.tile([128, QC], F32, tag="misc", name="dbc")
                    nc.tensor.matmul(dbc[:], ones65[64:65, :],
                                     av_sb[64:65, h * QC:(h + 1) * QC],
                                     start=True, stop=True)
                    nc.vector.reciprocal_approx_fast(dbc[:], dbc[:])
                    nc.vector.tensor_mul(ot_t[64 * h:64 * h + 64, m_t],
                                         av_sb[0:64, h * QC:(h + 1) * QC],
                                         dbc[64 * h:64 * h + 64, :])

            def wo_step(qc_w, tt, jc, i):
                if i == 0:
                    state["wo"] = miscp.tile([128, QC], F32, tag="misc", name="wop")
                wop = state["wo"]
                ot_w = ot_tiles[qc_w]
                tsl = slice(tt * 128, (tt + 1) * 128)
                for fb in (2 * i, 2 * i + 1):
                    nc.tensor.matmul(
                        wop[:], ot_w[:, fb, tsl], wo_sb[:, fb, jc * 512:(jc + 1) * 512],
                        start=(fb == 0), stop=(fb == NP - 1))
                if i == 1:
                    ostage = outp.tile([128, QC], BF16, tag="ostage", name="ostage")
                    nc.vector.tensor_copy(ostage[:], wop[:])
                    row = qc_w * QC + tt * 128
                    nc.sync.dma_start(
                        out[row:row + 128, jc * 512:(jc + 1) * 512], ostage[:])

            # ---- deadline-driven work queue (consumed FIFO, built sorted) ----
            work = []

            def push(deadline, cost, fn):
                work.append((deadline, cost, fn))

            def drain(g, budget=470):
                spent = 0
                while work:
                    d, cost, fn = work[0]
                    if d > g + 1 and spent + cost > budget:
                        break
                    work.pop(0)
                    fn()
                    spent += cost

            MARGIN = 2
            items = []
            for m in range(NP):
                for c in range(4):
                    if m == 0 and c == 0:
                        continue
                    items.append((gslot(m, 0, 4 * c) - MARGIN, "k", m, c))
                for c in range(4):
                    if m == 0 and c == 0:
                        continue
                    items.append((gslot(m, c, 0) - MARGIN, "q", m, c))
                for tt in range(NTT):
                    if m == 0 and tt < 2:
                        continue
                    items.append((gslot(m, 0, tt) - 1, "v", m, tt))
            items.sort(key=lambda it: it[0])
            for d, kind, m, x in items:
                if kind == "k":
                    for i in range(4):
                        push(d - (3 - i), 450, partial(kq_step, xk_sb, wk_sb, kt_sb, m, x, i))
                elif kind == "q":
                    for i in range(4):
                        push(d - (3 - i), 450, partial(kq_step, xq_sb, wq_sb, qt_sb, m, x, i))
                else:
                    for i in range(2):
                        push(d - (1 - i), 350, partial(v_step, m, x, i))

            # ---- prefix: Q m0 c0, K m0 c0 (matches DMA arrival order) ----
            for i in range(4):
                kq_step(xq_sb, wq_sb, qt_sb, 0, 0, i)
            for i in range(4):
                kq_step(xk_sb, wk_sb, kt_sb, 0, 0, i)

            def emit_scores(m, qc, kb):
                s = sp.tile([128, 1024], F32, tag="s", name="s")
                ksl = slice(kb * 128, (kb + 1) * 128)
                qsl = slice(qc * QC, (qc + 1) * QC)
                nc.tensor.matmul(s[:, 0:512], kt_sb[0:64, m, ksl], qt_sb[0:64, m, qsl],
                                 start=True, stop=True, tile_position=(0, 0))
                nc.tensor.matmul(s[:, 512:1024], kt_sb[64:128, m, ksl], qt_sb[64:128, m, qsl],
                                 start=True, stop=True, tile_position=(64, 0))
                return s

            cur = {}

            def emit_av(m, qc, kb, pt):
                if kb == 0:
                    cur["avA"] = avp.tile([128, QC], F32, tag="av", name="avA")
                    cur["avB"] = avp.tile([128, QC], F32, tag="av", name="avB")
                nc.tensor.matmul(cur["avA"][0:65, :], v_sb[:, kb, 2 * m, 0:65],
                                 pt[:, 0:512],
                                 start=(kb == 0), stop=(kb == NKB - 1))
                nc.tensor.matmul(cur["avB"][0:65, :], v_sb[:, kb, 2 * m + 1, 0:65],
                                 pt[:, 512:1024],
                                 start=(kb == 0), stop=(kb == NKB - 1))

            def evac_pair(m, qc):
                avA, avB = cur["avA"], cur["avB"]
                av_sb = smalls.tile([128, 1024], BF16, tag="av_sb", name="av_sb")
                nc.vector.tensor_copy(av_sb[0:65, 0:QC], avA[0:65, :])
                nc.vector.tensor_copy(av_sb[0:65, QC:2 * QC], avB[0:65, :])
                return (ot_tiles[qc], m, av_sb)

            SLOTS = [(m, qc, kb) for m in range(NP) for qc in range(NQC)
                     for kb in range(NKB)]
            s_cur = emit_scores(0, 0, 0)
            pending = None
            prev = None
            for t, (m, qc, kb) in enumerate(SLOTS):
                pt = ptp.tile([128, 1024], BF16, tag="pt", name="pt")
                nc.scalar.activation(pt[:], s_cur[:], EXP, scale=0.125)
                if t + 1 < len(SLOTS):
                    s_cur = emit_scores(*SLOTS[t + 1])
                if t == 0:
                    # V tt0/tt1 land between the first exp and AV(0)
                    for i in range(2):
                        v_step(0, 0, i)
                    for i in range(2):
                        v_step(0, 1, i)
                if prev is not None:
                    pm, pqc, pkb, ppt = prev
                    emit_av(pm, pqc, pkb, ppt)
                    if pkb == NKB - 1:
                        job = evac_pair(pm, pqc)
                        if pm < NP - 1 and pending is not None:
                            finish_pair(pending)
                        pending = job
                if kb == 0 and m == NP - 1:
                    # m3: flush the previous pair's norm mid-pair and pack the
                    # previous q chunk's W_o one step per slot
                    pj = pending
                    pending = None
                    g0 = gslot(m, qc, 0)
                    push(g0 + 2, 450, partial(finish_pair, pj))
                    if qc > 0:
                        for k in range(16):
                            j, jc, i = k // 4, (k // 2) % 2, k % 2
                            push(g0 + 3 + (11 * k) // 16, 450,
                                 partial(wo_step, qc - 1, j, jc, i))
                prev = (m, qc, kb, pt)
                drain(t)

            # drain: last AV, last pair's normalization, last q chunk's W_o
            pm, pqc, pkb, ppt = prev
            emit_av(pm, pqc, pkb, ppt)
            job = evac_pair(pm, pqc)
            drain(10 ** 9, budget=10 ** 9)
            finish_pair(job)
            for tt in range(4):
                for jc in range(2):
                    for i in range(2):
                        wo_step(NQC - 1, tt, jc, i)

    nc.compile()
    return nc


def _get_nc():
    global _nc_cache
    if _nc_cache is None:
        _nc_cache = build()
    return _nc_cache


def kernel(query, key, value, W_q, W_k, W_v, W_o):
    global last_results
    nc = _get_nc()
    bf = ml_dtypes.bfloat16

    in_maps = []
    xt = {}
    for b in range(B):
        xt[b] = {
            "xq": np.ascontiguousarray(query[b].T).astype(bf),
            "xk": np.ascontiguousarray(key[b].T).astype(bf),
            "xv": np.ascontiguousarray(value[b].T).astype(bf),
        }
    wmaps = []
    for hg in range(2):
        r = slice(hg * FPC, (hg + 1) * FPC)
        wmaps.append({
            "wq": np.ascontiguousarray(W_q[r, :].T).astype(bf),
            "wk": np.ascontiguousarray(W_k[r, :].T).astype(bf),
            "wv": np.ascontiguousarray(W_v[r, :].T).astype(bf),
            "wo": np.ascontiguousarray(W_o[:, r].T).astype(bf),
        })
    for c in range(8):
        b, hg = c // 2, c % 2
        in_maps.append({**xt[b], **wmaps[hg]})

    res = run_bass_kernel_spmd(
        nc, in_maps, core_ids=list(range(8)),
        trace=bool(os.environ.get("BASS_KERNEL_TRACE")))
    last_results = res

    out = np.empty((B, S, D), np.float32)
    for b in range(B):
        out[b] = (res.results[2 * b]["out"].astype(np.float32)
                  + res.results[2 * b + 1]["out"].astype(np.float32))
    return out
